# revision 1
# baseline (speedup 1.0000x reference)
"""Trainium2 Bass kernel for a 16-head decoder self-attention block.

Reference computation (B=2, S=2048, E=2048, H=16, D=128):
    qkv = X @ W_qkv.T + b_qkv ; RoPE(Q, K) ; attn = softmax(QK^T/sqrt(D) + mask)
    out = (attn @ V reshaped) @ W_o.T + b_o

Sharding over 8 NeuronCores: data parallel over batch (2) x tensor parallel
over 4 head-groups of 4 heads each. Each core computes its group's qkv
projection, attention, and a partial (rank-512) slice of the output
projection; the host sums the 4 partials per batch element.

All matmuls run as float32r (TF32-like fast fp32 path on the PE array).

Per-core dataflow (all matmuls contract over the partition dim):
  phase 1: qkv^T[m,:] = W1[:,m-tile]^T @ X^T, streamed over two S-halves.
           Q/K tiles get bias+RoPE fused on evacuation (RoPE's dim shuffle
           is a signed permutation matmul); V tiles get bias only and stay
           transposed (d on partitions).
  phase 2: per head: scoresT = K^T_tiles^T @ Q^T (sk on partitions), fused
           exp(scale*x + mask_bias) on ScalarE, then outT += V_tile^T @ exp
           and denom += ones^T @ exp accumulate in PSUM; normalize with a
           reciprocal multiply. V tiles come from PE-transposing V^T.
  phase 3: partial^T = W2^T @ A^T with b_o added on one head-group.
"""

import math
import sys

import numpy as np

sys.path.insert(0, "/opt/trn_rl_repo")

B, S, E = 2, 2048, 2048
H, D = 16, 128
NCORES = 8
NGROUP = 4          # head groups (tensor parallel)
HPG = H // NGROUP   # heads per group = 4
GE = HPG * D        # group embed width = 512
KT = E // 128       # contraction tiles over E = 16
ST = S // 128       # sequence tiles = 16
MT = 3 * HPG        # qkv m-tiles per core = 12
SCALE = 1.0 / math.sqrt(D)

_CACHE = {}


def _build():
    """Build + compile the per-core Bass program (same program, all cores)."""
    import concourse.bacc as bacc
    import concourse.mybir as mybir
    import concourse.tile as tile

    F32 = mybir.dt.float32
    F32R = mybir.dt.float32r
    EXP = mybir.ActivationFunctionType.Exp
    IDENT = mybir.ActivationFunctionType.Identity

    nc = bacc.Bacc("TRN2", target_bir_lowering=False, debug=False)

    xt = nc.dram_tensor("xt", [E, S], F32, kind="ExternalInput").ap()        # X[b].T
    w1t = nc.dram_tensor("w1t", [MT * 128, E], F32, kind="ExternalInput").ap()
    w2t = nc.dram_tensor("w2t", [KT * 128, GE], F32, kind="ExternalInput").ap()
    bqkv = nc.dram_tensor("bqkv", [128, MT], F32, kind="ExternalInput").ap()
    bo = nc.dram_tensor("bo", [128, KT], F32, kind="ExternalInput").ap()
    mb = nc.dram_tensor("mb", [128, ST], F32, kind="ExternalInput").ap()     # mask bias
    cosx = nc.dram_tensor("cosx", [128, S], F32, kind="ExternalInput").ap()
    sinx = nc.dram_tensor("sinx", [128, S], F32, kind="ExternalInput").ap()
    ones = nc.dram_tensor("ones", [128, 128], F32, kind="ExternalInput").ap()
    perm = nc.dram_tensor("perm", [128, 128], F32, kind="ExternalInput").ap()
    ident = nc.dram_tensor("ident", [128, 128], F32, kind="ExternalInput").ap()
    pout = nc.dram_tensor("pout", [E, S], F32, kind="ExternalOutput").ap()

    with tile.TileContext(nc) as tc:
        with tc.tile_pool(name="dram", bufs=1, space="DRAM") as dpool, \
             tc.tile_pool(name="small", bufs=1) as spool:
            # qkv^T scratch: m 0..7 = rope'd Q/K (f32), m 8..11 = V^T (f32r)
            qkd = [dpool.tile([128, S], F32 if m < 2 * HPG else F32R,
                              tag=f"qkd{m}", name=f"qkd{m}")
                   for m in range(MT)]

            ones_sb = spool.tile([128, 128], F32R, tag="ones")
            nc.sync.dma_start(ones_sb[:], ones.bitcast(F32R))
            mb_sb = spool.tile([128, ST], F32, tag="mb")
            nc.sync.dma_start(mb_sb[:], mb)
            bo_sb = spool.tile([128, KT], F32, tag="bo")
            nc.sync.dma_start(bo_sb[:], bo)
            id_sb = spool.tile([128, 128], F32, tag="id")
            nc.sync.dma_start(id_sb[:], ident)

            # ---------------- Phase 1: qkv projections ----------------
            vtp = tc.alloc_tile_pool(name="vt", bufs=2)
            with tc.tile_pool(name="xt", bufs=1) as xpool, \
                 tc.tile_pool(name="trig", bufs=1) as trig, \
                 tc.tile_pool(name="w1", bufs=3) as w1p, \
                 tc.tile_pool(name="qkps", bufs=2, space="PSUM") as qkps, \
                 tc.tile_pool(name="qkps2", bufs=2, space="PSUM") as qkps2, \
                 tc.tile_pool(name="qbp", bufs=2) as qbp, \
                 tc.tile_pool(name="rap", bufs=1) as rap, \
                 tc.tile_pool(name="stp", bufs=2) as stp:
                cos_sb = trig.tile([128, S], F32, tag="cos")
                sin_sb = trig.tile([128, S], F32, tag="sin")
                bq_sb = trig.tile([128, MT], F32, tag="bq")
                perm_sb = trig.tile([128, 128], F32R, tag="perm")

                # one fat DMA per m-column of W1 (host pre-packed so the 16
                # k-tiles are contiguous); interleave with the X^T stream
                wt = [None] * MT
                wt[0] = w1p.tile([128, E], F32R, tag="w1", name="w1_0")
                nc.sync.dma_start(wt[0][:], w1t[0:128, :].bitcast(F32R))
                xts = []
                for k in range(KT):
                    t = xpool.tile([128, S], F32R, tag=f"xt{k}", name=f"xts{k}")
                    nc.sync.dma_start(t[:], xt[k * 128:(k + 1) * 128, :].bitcast(F32R))
                    xts.append(t)
                    if k == 0:
                        wt[1] = w1p.tile([128, E], F32R, tag="w1", name="w1_1")
                        nc.sync.dma_start(wt[1][:], w1t[128:256, :].bitcast(F32R))
                    elif k == 1:
                        nc.sync.dma_start(cos_sb[:], cosx)
                    elif k == 2:
                        nc.sync.dma_start(sin_sb[:], sinx)
                    elif k == 3:
                        nc.sync.dma_start(bq_sb[:], bqkv)
                        nc.sync.dma_start(perm_sb[:], perm.bitcast(F32R))

                vt_h0 = None
                for m in range(MT):
                    if m + 2 < MT:
                        w = w1p.tile([128, E], F32R, tag="w1", name=f"w1_{m + 2}")
                        nc.sync.dma_start(
                            w[:], w1t[(m + 2) * 128:(m + 3) * 128, :].bitcast(F32R))
                        wt[m + 2] = w
                    for half in range(2):
                        hs = slice(half * 1024, (half + 1) * 1024)
                        ps = qkps.tile([128, 1024], F32, tag="ps")
                        for k in range(KT):
                            for ns in range(2):
                                sl = slice(ns * 512, (ns + 1) * 512)
                                nc.tensor.matmul(
                                    ps[:, sl], wt[m][:, k * 128:(k + 1) * 128],
                                    xts[k][:, half * 1024 + ns * 512:
                                            half * 1024 + (ns + 1) * 512],
                                    start=(k == 0), stop=(k == KT - 1))
                        qb = qbp.tile([128, 1024], F32R, tag="qb")
                        nc.scalar.activation(qb[:], ps[:], IDENT,
                                             bias=bq_sb[:, m:m + 1], scale=1.0)
                        if m < 2 * HPG:
                            # RoPE: trans() is a signed dim-permutation matmul
                            ps2 = qkps2.tile([128, 1024], F32, tag="ps2")
                            for ns in range(2):
                                sl = slice(ns * 512, (ns + 1) * 512)
                                nc.tensor.matmul(ps2[:, sl], perm_sb[:], qb[:, sl],
                                                 start=True, stop=True)
                            ra = rap.tile([128, 1024], F32, tag="ra")
                            nc.vector.tensor_mul(ra[:], qb[:].bitcast(F32),
                                                 cos_sb[:, hs])
                            st = stp.tile([128, 1024], F32, tag="st")
                            nc.vector.tensor_mul(st[:], ps2[:], sin_sb[:, hs])
                            nc.vector.tensor_add(st[:], st[:], ra[:])
                            nc.sync.dma_start(qkd[m][:, hs], st[:])
                        else:
                            nc.sync.dma_start(qkd[m][:, hs], qb[:])
                    if m == 2 * HPG:
                        # prefetch head 0's V^T as soon as it is complete
                        vt_h0 = vtp.tile([128, S], F32R, tag="vt", name="vt_h0")
                        nc.sync.dma_start(vt_h0[:], qkd[2 * HPG][:])

            # ---------------- Phase 2: attention per head ----------------
            NQC = 2            # q-chunks of 1024 (amortizes ACT exp overhead)
            QW = S // NQC
            with tc.tile_pool(name="at", bufs=1) as atp:
                at_tiles = [atp.tile([128, S], F32R, tag=f"at{h}", name=f"at{h}")
                            for h in range(HPG)]
                with tc.tile_pool(name="qk", bufs=2) as qkp, \
                     tc.tile_pool(name="vh", bufs=2) as vhp, \
                     tc.tile_pool(name="ex", bufs=4) as exp_pool, \
                     tc.tile_pool(name="rc", bufs=2) as rcp, \
                     tc.tile_pool(name="oev", bufs=2) as oev, \
                     tc.tile_pool(name="pss", bufs=2, space="PSUM") as pss_pool, \
                     tc.tile_pool(name="pso", bufs=1, space="PSUM") as pso_pool, \
                     tc.tile_pool(name="psd", bufs=1, space="PSUM") as psd_pool:
                    for h in range(HPG):
                        qt = qkp.tile([128, S], F32R, tag="qt")
                        nc.sync.dma_start(qt[:], qkd[h][:].bitcast(F32R))
                        kt_ = qkp.tile([128, S], F32R, tag="kt")
                        nc.sync.dma_start(kt_[:], qkd[HPG + h][:].bitcast(F32R))
                        if h == 0:
                            vt = vt_h0
                        else:
                            vt = vtp.tile([128, S], F32R, tag="vt",
                                          name=f"vt_h{h}")
                            nc.sync.dma_start(vt[:], qkd[2 * HPG + h][:])
                        # V tiles [sk, d] from V^T via PE transpose
                        vh = []
                        for ms in range(ST):
                            tp = pss_pool.tile([128, QW], F32, tag="pss",
                                               name=f"tp{ms}")
                            nc.tensor.transpose(
                                tp[:, 0:128],
                                vt[:, ms * 128:(ms + 1) * 128].bitcast(F32),
                                id_sb[:])
                            t = vhp.tile([128, 128], F32R, tag=f"vh{ms}",
                                         name=f"vh{ms}")
                            nc.vector.tensor_copy(t[:], tp[:, 0:128])
                            vh.append(t)
                        # flattened (qc, ms) stream with the V/denom
                        # accumulation trailing one step, so the PE pipeline
                        # never drains at chunk boundaries
                        pso = psd = None
                        prev = None
                        for step in range(NQC * ST + 1):
                            if step < NQC * ST:
                                qc, ms = divmod(step, ST)
                                if ms == 0:
                                    cur_pso = pso_pool.tile([128, QW], F32,
                                                            tag="pso",
                                                            name=f"pso{h}_{qc}")
                                    cur_psd = psd_pool.tile([128, QW], F32,
                                                            tag="psd",
                                                            name=f"psd{h}_{qc}")
                                pss = pss_pool.tile([128, QW], F32, tag="pss")
                                for ns in range(2):
                                    sl = slice(ns * 512, (ns + 1) * 512)
                                    nc.tensor.matmul(
                                        pss[:, sl],
                                        kt_[:, ms * 128:(ms + 1) * 128],
                                        qt[:, qc * QW + ns * 512:
                                           qc * QW + (ns + 1) * 512],
                                        start=True, stop=True)
                                ex = exp_pool.tile([128, QW], F32R, tag="ex")
                                nc.scalar.activation(ex[:], pss[:], EXP,
                                                     bias=mb_sb[:, ms:ms + 1],
                                                     scale=SCALE)
                            if prev is not None:
                                pms, pex, ppso, ppsd = prev
                                for ns in range(2):
                                    sl = slice(ns * 512, (ns + 1) * 512)
                                    nc.tensor.matmul(ppso[:, sl], vh[pms][:],
                                                     pex[:, sl],
                                                     start=(pms == 0),
                                                     stop=(pms == ST - 1))
                                    nc.tensor.matmul(ppsd[:, sl], ones_sb[:],
                                                     pex[:, sl],
                                                     start=(pms == 0),
                                                     stop=(pms == ST - 1))
                                if pms == ST - 1:
                                    pqc = prev_qc
                                    qsl = slice(pqc * QW, (pqc + 1) * QW)
                                    dsb = oev.tile([128, QW], F32, tag="dsb")
                                    nc.vector.tensor_copy(dsb[:], ppsd[:])
                                    osb = oev.tile([128, QW], F32, tag="osb")
                                    nc.vector.tensor_copy(osb[:], ppso[:])
                                    rc = rcp.tile([128, QW], F32, tag="rc")
                                    nc.vector.reciprocal_approx_fast(rc[:], dsb[:])
                                    nc.vector.tensor_mul(at_tiles[h][:, qsl],
                                                         osb[:], rc[:])
                            if step < NQC * ST:
                                prev = (ms, ex, cur_pso, cur_psd)
                                prev_qc = qc

                # ---------------- Phase 3: output projection (partial) ----------------
                with tc.tile_pool(name="w2", bufs=3) as w2p, \
                     tc.tile_pool(name="ops", bufs=3, space="PSUM") as ops_pool, \
                     tc.tile_pool(name="ost", bufs=3) as ost:
                    w2s = [None] * KT
                    w2s[0] = w2p.tile([128, GE], F32R, tag="w2", name="w2_0")
                    nc.sync.dma_start(w2s[0][:], w2t[0:128, :].bitcast(F32R))
                    for m in range(KT):
                        if m + 1 < KT:
                            w = w2p.tile([128, GE], F32R, tag="w2",
                                         name=f"w2_{m + 1}")
                            nc.sync.dma_start(
                                w[:], w2t[(m + 1) * 128:(m + 2) * 128,
                                          :].bitcast(F32R))
                            w2s[m + 1] = w
                        for qc in range(2):
                            ps = ops_pool.tile([128, 1024], F32, tag="ps")
                            for k in range(HPG):
                                for ns in range(2):
                                    sl_o = slice(ns * 512, (ns + 1) * 512)
                                    sl_i = slice(qc * 1024 + ns * 512,
                                                 qc * 1024 + (ns + 1) * 512)
                                    nc.tensor.matmul(
                                        ps[:, sl_o],
                                        w2s[m][:, k * 128:(k + 1) * 128],
                                        at_tiles[k][:, sl_i],
                                        start=(k == 0),
                                        stop=(k == HPG - 1))
                            st = ost.tile([128, 1024], F32, tag="st")
                            nc.scalar.activation(st[:], ps[:], IDENT,
                                                 bias=bo_sb[:, m:m + 1], scale=1.0)
                            nc.sync.dma_start(
                                pout[m * 128:(m + 1) * 128,
                                     qc * 1024:(qc + 1) * 1024], st[:])

            vtp.release()

    nc.compile()
    return nc


def _rope_tables():
    # Bug-faithful to the reference: exponent divides by EMB_DIM, not head_dim.
    angle = 1.0 / np.power(10000.0, np.arange(0, D, 2, dtype=np.float64) / E)
    t = np.arange(S, dtype=np.float64)
    freqs = np.repeat(t[:, None] * angle[None, :], 2, axis=-1)  # [S, D]
    return np.cos(freqs).astype(np.float32), np.sin(freqs).astype(np.float32)


def _prep_inputs(X, mask, W_qkv, b_qkv, W_o, b_o):
    """Build the 8 per-core input maps."""
    X = np.ascontiguousarray(np.asarray(X, dtype=np.float32))
    mask = np.asarray(mask)
    W_qkv = np.asarray(W_qkv, dtype=np.float32)
    b_qkv = np.asarray(b_qkv, dtype=np.float32)
    W_o = np.asarray(W_o, dtype=np.float32)
    b_o = np.asarray(b_o, dtype=np.float32)

    cos, sin = _rope_tables()
    cosx = np.ascontiguousarray(cos.T)                      # [D, S]
    sinx = np.ascontiguousarray(sin.T)                      # [D, S]
    ones = np.ones((128, 128), dtype=np.float32)
    ident = np.eye(128, dtype=np.float32)
    # trans(q)[j] = -q[2j+1] (j<64), +q[2j-128] (j>=64), as lhsT: permT[d, j]
    permT = np.zeros((128, 128), dtype=np.float32)
    for j in range(64):
        permT[2 * j + 1, j] = -1.0
    for j in range(64, 128):
        permT[2 * (j - 64), j] = 1.0

    xts = [np.ascontiguousarray(X[b].T) for b in range(B)]
    mbs = []
    for b in range(B):
        m = np.where(mask[b] == 0, np.float32(-1e9), np.float32(0.0)).astype(np.float32)
        mbs.append(np.ascontiguousarray(m.reshape(ST, 128).T))
    bo_t = np.ascontiguousarray(b_o.reshape(KT, 128).T)
    bo_z = np.zeros_like(bo_t)

    in_maps = []
    for c in range(NCORES):
        b, g = divmod(c, NGROUP)
        qs = slice(g * GE, (g + 1) * GE)
        ks = slice(E + g * GE, E + (g + 1) * GE)
        vs = slice(2 * E + g * GE, 2 * E + (g + 1) * GE)
        w1 = np.concatenate([W_qkv[qs], W_qkv[ks], W_qkv[vs]], axis=0)  # [1536, E]
        bqkv_v = np.concatenate([b_qkv[qs], b_qkv[ks], b_qkv[vs]])      # [1536]
        # pack W1^T so each m-column's 16 k-tiles are contiguous:
        # w1p[m][e_loc, k*128+col] = W1^T[k*128+e_loc, m*128+col]
        w1tt = np.ascontiguousarray(w1.T)                       # [E, 1536]
        w1pk = w1tt.reshape(KT, 128, MT, 128).transpose(2, 1, 0, 3).reshape(
            MT * 128, E)
        w2tt = np.ascontiguousarray(W_o[:, g * GE:(g + 1) * GE].T)  # [512, E]
        w2pk = w2tt.reshape(HPG, 128, KT, 128).transpose(2, 1, 0, 3).reshape(
            KT * 128, GE)
        in_maps.append({
            "xt": xts[b],
            "w1t": np.ascontiguousarray(w1pk),
            "w2t": np.ascontiguousarray(w2pk),
            "bqkv": np.ascontiguousarray(bqkv_v.reshape(MT, 128).T),
            "bo": bo_t if g == 0 else bo_z,
            "mb": mbs[b],
            "cosx": cosx,
            "sinx": sinx,
            "ones": ones,
            "perm": permT,
            "ident": ident,
        })
    return in_maps


def kernel(X, mask, W_qkv, b_qkv, W_o, b_o, _trace=False):
    from concourse.bass_utils import run_bass_kernel_spmd

    if "nc" not in _CACHE:
        _CACHE["nc"] = _build()
    nc = _CACHE["nc"]

    in_maps = _prep_inputs(X, mask, W_qkv, b_qkv, W_o, b_o)
    res = run_bass_kernel_spmd(nc, in_maps, core_ids=list(range(NCORES)),
                               trace=_trace)
    _CACHE["last_result"] = res

    out = np.empty((B, S, E), dtype=np.float32)
    for b in range(B):
        acc = res.results[b * NGROUP]["pout"].copy()
        for g in range(1, NGROUP):
            acc += res.results[b * NGROUP + g]["pout"]
        out[b] = acc.T
    return out

